# revision 1
# baseline (speedup 1.0000x reference)
"""Multi-head self-attention (B=8, S=1024, D=768, H=12) on 8 trn2 cores.

Sharding: data-parallel over batch — core b computes attention for Q[b].
No collectives. Host pre-transposes Q (to X^T) and the weights (to W^T,
i.e. [d_in, d_out]) so every on-device matmul contracts over the
partition dim with zero on-device transposes; the device returns
ctx^T [768, 1024] which the host transposes back.

Device layout (per core):
  qt   [768,1024] = Q[b]^T                    (d_in on partitions)
  w*t  [768, 768] = W^T                        (d_in on partitions)
  qT/kT [768,1024] = (XW^T+b)^T                (d_out on partitions)
  v    packed [128, 8, 12*65]: per head 64 v-columns + a ones column
       (ones row makes the ctx matmul also emit the softmax denominator)
  scores^T [s_k, s_q] per head: K=64 matmul; exp via ACT (scale=1/8 fused)
  ctx^T accum over s_k chunks: [65, 512] PSUM; row 64 = sum(exp) = Z
  normalize: ctx^T * (1/Z), 1/Z replicated across partitions via a
  DRAM round-trip DMA (SBUF sources cannot partition-broadcast)
"""

import ml_dtypes
import numpy as np

import concourse.bass as bass
import concourse.mybir as mybir
import concourse.tile as tile
from concourse.bass_utils import run_bass_kernel_spmd

F32 = mybir.dt.float32
BF16 = mybir.dt.bfloat16

S = 1024
D = 768
H = 12
DK = 64
KC = D // 128   # 6 contraction chunks
MC = D // 128   # 6 output-row chunks
SC = S // 128   # 8 sequence chunks
NSQ = S // 512  # 2 query-column chunks
SCALE = 1.0 / np.sqrt(DK)
VROW = 65       # 64 v columns + 1 ones column per head
KGROUPS = [(0, 3), (3, 3), (6, 2)]   # s_k chunk groups for scores/exp
KC2G = {g0 + i: (g, i) for g, (g0, glen) in enumerate(KGROUPS)
        for i in range(glen)}


def _split_excess_waits(nc, max_waits=1):
    """This container's walrus encodes at most one sem-wait per
    instruction; spread extra waits onto EventSemaphore instructions."""
    for fn in nc.m.functions:
        for bb in fn.blocks:
            out = []
            for ins in bb.instructions:
                si = getattr(ins, "sync_info", None)
                ow = list(si.on_wait) if (si is not None and si.on_wait) else []
                if len(ow) > max_waits:
                    head, tail = ow[:-max_waits], ow[-max_waits:]
                    for j in range(0, len(head), max_waits):
                        ev = mybir.InstEventSemaphore(
                            name=f"evsplit-{ins.name}-{j}", ins=[], outs=[])
                        ev.engine = ins.engine
                        ev.sync_info = mybir.SyncInfo(
                            on_wait=head[j:j + max_waits], on_update=[])
                        out.append(ev)
                    ins.sync_info = mybir.SyncInfo(
                        on_wait=tail, on_update=list(si.on_update))
                out.append(ins)
            bb.instructions = out


def build_nc():
    nc = bass.Bass(trn_type="TRN2")

    qt = nc.dram_tensor("qt", [D, S], BF16, kind="ExternalInput").ap()
    wqt = nc.dram_tensor("wqt", [D, D], BF16, kind="ExternalInput").ap()
    wkt = nc.dram_tensor("wkt", [D, D], BF16, kind="ExternalInput").ap()
    wvt = nc.dram_tensor("wvt", [D, D], BF16, kind="ExternalInput").ap()
    bq = nc.dram_tensor("bq", [D], F32, kind="ExternalInput").ap()
    bk = nc.dram_tensor("bk", [D], F32, kind="ExternalInput").ap()
    bv = nc.dram_tensor("bv", [D], F32, kind="ExternalInput").ap()
    ctxt = nc.dram_tensor("ctxt", [D, S], F32, kind="ExternalOutput").ap()

    with tile.TileContext(nc) as tc:
        with (
            tc.tile_pool(name="singles", bufs=1) as singles,
            tc.tile_pool(name="psA", bufs=2, space="PSUM") as psA,
            tc.tile_pool(name="psP", bufs=2, space="PSUM") as psP,
            tc.tile_pool(name="expp", bufs=4) as expp,
            tc.tile_pool(name="ctop", bufs=4) as ctop,
            tc.tile_pool(name="recp", bufs=4) as recp,
            tc.tile_pool(name="recd", bufs=3, space="DRAM") as recd,
        ):
            # ---- persistent SBUF arrays --------------------------------
            qt_sb = singles.tile([128, KC, S], BF16)      # X^T
            wq_sb = singles.tile([128, KC, D], BF16)      # Wq^T
            wk_sb = singles.tile([128, KC, D], BF16)
            wv_sb = singles.tile([128, KC, D], BF16)
            qT_sb = singles.tile([128, MC, S], BF16)      # q^T
            kT_sb = singles.tile([128, MC, S], BF16)
            v_sb = singles.tile([128, SC, H * VROW], BF16)
            bq_sb = singles.tile([128, MC], F32)
            bk_sb = singles.tile([128, MC], F32)
            bvb_sb = singles.tile([128, H, DK], BF16)     # bv bcast over partitions

            # ---- input DMAs, ordered so q/k mc=0 projections start ASAP:
            # qt chunk-by-chunk, first column-slice of wq/wk, then the rest.
            nc.sync.dma_start(out=bq_sb, in_=bq.rearrange("(c p) -> p c", p=128))
            nc.scalar.dma_start(out=bk_sb,
                                in_=bk.rearrange("(c p) -> p c", p=128))
            qtr = qt.rearrange("(c p) s -> p c s", p=128)
            dma_engs = [nc.sync, nc.scalar]
            for kc in range(KC):
                dma_engs[kc % 2].dma_start(out=qt_sb[:, kc, :],
                                           in_=qtr[:, kc, :])
            wqr = wqt.rearrange("(c p) n -> p c n", p=128)
            wkr = wkt.rearrange("(c p) n -> p c n", p=128)
            nc.sync.dma_start(out=wq_sb[:, :, 0:128], in_=wqr[:, :, 0:128])
            nc.scalar.dma_start(out=wk_sb[:, :, 0:128], in_=wkr[:, :, 0:128])
            wvr = wvt.rearrange("(c p) n -> p c n", p=128)
            nc.sync.dma_start(out=wv_sb[:, :, 0:384], in_=wvr[:, :, 0:384])
            nc.scalar.dma_start(out=wv_sb[:, :, 384:768], in_=wvr[:, :, 384:768])
            nc.sync.dma_start(out=wq_sb[:, :, 128:768], in_=wqr[:, :, 128:768])
            nc.scalar.dma_start(out=wk_sb[:, :, 128:768], in_=wkr[:, :, 128:768])
            bv_bcast = bass.AP(tensor=bv.tensor, offset=bv.offset,
                               ap=[[0, 128], [DK, H], [1, DK]])
            nc.gpsimd.dma_start(out=bvb_sb, in_=bv_bcast)  # casts f32->bf16

            # ones columns of v (col 64 of each 65-wide head group)
            v4 = v_sb.rearrange("p s (h c) -> p s h c", c=VROW)
            nc.vector.memset(v4[:, :, :, DK:DK + 1], 1.0)

            # ---- emission helpers (emission order == scheduler priority) --

            def proj_qk(mc):
                """q^T and k^T rows for head pair mc."""
                for (w_sb, b_sb, o_sb) in ((wq_sb, bq_sb, qT_sb),
                                           (wk_sb, bk_sb, kT_sb)):
                    for n in range(NSQ):
                        ps = psP.tile([128, 512], F32, tag="proj",
                                      name=f"pj_{mc}_{n}")
                        for kc in range(KC):
                            nc.tensor.matmul(
                                ps,
                                lhsT=w_sb[:, kc, mc * 128:(mc + 1) * 128],
                                rhs=qt_sb[:, kc, n * 512:(n + 1) * 512],
                                start=(kc == 0), stop=(kc == KC - 1),
                            )
                        nc.vector.tensor_scalar_add(
                            out=o_sb[:, mc, n * 512:(n + 1) * 512],
                            in0=ps,
                            scalar1=b_sb[:, mc:mc + 1],
                        )

            def proj_v():
                """v[s, d] = X @ Wv^T + bv, packed 65-strided with ones col."""
                for sc in range(SC):
                    for n in range(2):       # d_out in two 384 chunks
                        ps = psP.tile([128, 512], F32, tag="proj",
                                      name=f"pv_{sc}_{n}")
                        for kc in range(KC):
                            nc.tensor.matmul(
                                ps[:, 0:384],
                                lhsT=qt_sb[:, kc, sc * 128:(sc + 1) * 128],
                                rhs=wv_sb[:, kc, n * 384:(n + 1) * 384],
                                start=(kc == 0), stop=(kc == KC - 1),
                            )
                        nc.vector.tensor_add(
                            out=v4[:, sc, 6 * n:6 * n + 6, 0:DK],
                            in0=ps[:, 0:384].rearrange("p (h c) -> p h c", c=DK),
                            in1=bvb_sb[:, 6 * n:6 * n + 6, :],
                        )

            def sc_exp(mc, j):
                """Scores^T + exp for both heads of pair mc, query cols j.
                The two heads' K=64 matmuls are emitted back-to-back from
                partition bases 0/64 so the PE runs them concurrently in
                distinct row groups (row tiling)."""
                # kc groups of (3,3,2): bigger ACT instructions amortize
                # the per-instruction ACT overhead; one exp tile per
                # (head, group) so ctx matmuls unblock per group
                exp_ts = [[expp.tile([128, glen, 512], BF16,
                                     tag=f"exp{hh}g{g}",
                                     name=f"exp_{mc}_{j}_{hh}_{g}")
                           for g, (g0, glen) in enumerate(KGROUPS)]
                          for hh in range(2)]
                for g, (g0, glen) in enumerate(KGROUPS):
                    # both heads draw from one 2-slot rotation (6 banks):
                    # ACT drains slot A while the PE refills slot B
                    pss = [psA.tile([128, 3, 512], F32, tag="sc",
                                    name=f"sc_{mc}_{j}_{g}_{hh}")
                           for hh in range(2)]
                    for i in range(glen):
                        kc2 = g0 + i
                        for hh in range(2):
                            pb = hh * DK
                            nc.tensor.matmul(
                                pss[hh][:, i, :],
                                lhsT=kT_sb[pb:pb + DK, mc,
                                           kc2 * 128:(kc2 + 1) * 128],
                                rhs=qT_sb[pb:pb + DK, mc,
                                          j * 512:(j + 1) * 512],
                                start=True, stop=True,
                            )
                    for hh in range(2):
                        nc.scalar.activation(
                            out=exp_ts[hh][g],
                            in_=pss[hh][:, 0:glen, :],
                            func=mybir.ActivationFunctionType.Exp,
                            scale=float(SCALE),
                        )
                return exp_ts

            def ctx(mc, j, exp_ts):
                """ctx^T + normalization for both heads of pair mc."""
                for hh in range(2):
                    h = 2 * mc + hh
                    exp_t = exp_ts[hh]
                    # ctx^T (rows 0:64) + Z (row 64), accumulated over s_k
                    psc = psP.tile([128, 512], F32, tag="proj",
                                   name=f"psc_{mc}_{j}_{hh}")
                    for kc2 in range(SC):
                        g, i = KC2G[kc2]
                        nc.tensor.matmul(
                            psc[0:VROW, :],
                            lhsT=v_sb[:, kc2, h * VROW:(h + 1) * VROW],
                            rhs=exp_t[g][:, i, :],
                            start=(kc2 == 0), stop=(kc2 == SC - 1),
                        )
                    # free the PSUM bank quickly: copy ctx+Z to SBUF, then
                    # run the recip/broadcast/normalize chain off SBUF
                    cts = ctop.tile([VROW, 512], F32, tag="cts",
                                    name=f"cts_{mc}_{j}_{hh}")
                    nc.vector.tensor_copy(out=cts, in_=psc[0:VROW, :])
                    rec = recp.tile([1, 512], F32, tag="rec",
                                    name=f"rec_{mc}_{j}_{hh}")
                    nc.vector.reciprocal(out=rec, in_=cts[DK:DK + 1, :])
                    # SBUF->SBUF partition-broadcast is not allowed; bounce
                    # the 2KB row through DRAM and read it back replicated
                    # across 64 partitions.
                    recdram = recd.tile([1, 512], F32, tag="recd",
                                        name=f"recd_{mc}_{j}_{hh}")
                    nc.sync.dma_start(out=recdram, in_=rec)
                    recb = recp.tile([64, 512], F32, tag="recb",
                                     name=f"recb_{mc}_{j}_{hh}")
                    nc.sync.dma_start(out=recb,
                                      in_=recdram.to_broadcast([64, 512]))
                    cto = ctop.tile([64, 512], F32, tag="cto",
                                    name=f"cto_{mc}_{j}_{hh}")
                    nc.vector.tensor_mul(out=cto, in0=cts[0:DK, :], in1=recb)
                    nc.sync.dma_start(
                        out=ctxt[h * DK:(h + 1) * DK, j * 512:(j + 1) * 512],
                        in_=cto)

            # ---- software pipeline ----------------------------------------
            # qk(0) + scores/exp(0) first so ACT starts ~5us in; v overlaps
            # pair-0 exps; from then on ctx(mc) is deferred past qk(mc+1) and
            # sc_exp(mc+1) emission so the PE always has high-priority work
            # while ACT chews the previous pair's exps.
            proj_qk(0)
            exps = {j: sc_exp(0, j) for j in range(NSQ)}
            proj_v()
            for mc in range(1, MC):
                proj_qk(mc)
                for j in range(NSQ):
                    ctx(mc - 1, j, exps[j])
                if mc < MC - 1:
                    exps = {j: sc_exp(mc, j) for j in range(NSQ)}
                else:
                    for j in range(NSQ):
                        e = sc_exp(mc, j)
                        ctx(mc, j, e)

    _split_excess_waits(nc)
    return nc


_NC_CACHE = None


def _get_nc():
    global _NC_CACHE
    if _NC_CACHE is None:
        _NC_CACHE = build_nc()
    return _NC_CACHE


def kernel(Q, Wq, bq, Wk, bk, Wv, bv):
    BF = ml_dtypes.bfloat16
    Q = np.asarray(Q, np.float32)
    wqt = np.ascontiguousarray(np.asarray(Wq, np.float32).T.astype(BF))
    wkt = np.ascontiguousarray(np.asarray(Wk, np.float32).T.astype(BF))
    wvt = np.ascontiguousarray(np.asarray(Wv, np.float32).T.astype(BF))
    bq = np.ascontiguousarray(np.asarray(bq, np.float32))
    bk = np.ascontiguousarray(np.asarray(bk, np.float32))
    bv = np.ascontiguousarray(np.asarray(bv, np.float32))

    nc = _get_nc()
    in_maps = []
    for b in range(Q.shape[0]):
        in_maps.append({
            "qt": np.ascontiguousarray(Q[b].T.astype(BF)),
            "wqt": wqt, "wkt": wkt, "wvt": wvt,
            "bq": bq, "bk": bk, "bv": bv,
        })
    res = run_bass_kernel_spmd(nc, in_maps, core_ids=list(range(len(in_maps))))
    out = np.stack([np.ascontiguousarray(r["ctxt"].T) for r in res.results])
    return out

